# revision 10
# baseline (speedup 1.0000x reference)
"""Causal self-attention (B=2, T=2048, D=1024, H=16, hd=64) on 8 TRN2 cores.

Sharding: 2 batches x 4 head-groups (4 heads each). Each core computes the
full pipeline for its (batch, head-group): qkv projection (transposed
layout), causal attention, and its partial output projection. The host sums
the 4 per-batch partials (tensor-parallel reduce) and adds bproj.

Device-side layout notes:
 - x is passed pre-transposed (xT [D, T]) so the qkv projection can contract
   over D on the partition dimension.
 - Scores are computed transposed (St = k @ qT, [k_tok, q_tok]) so softmax's
   exp feeds straight into att@v as the moving operand without transposes.
 - Softmax has no max-subtraction (scores are O(6) here, exp is safe) and the
   denominator is produced by augmenting v with a ones column (M=65 matmul).
 - The 1/sqrt(hd) scale is folded into Wq/bq on the host.
"""

import sys

sys.path.insert(0, "/opt/trn_rl_repo")

import numpy as np

B, T, D = 2, 2048, 1024
N_HEAD = 16
HD = 64  # head dim
HPC = 4  # heads per core
N_CORES = 8

P = 128
NJ = 512  # q-slice width
JT = T // NJ  # 4 q-slices
KT = D // P  # 8 contraction tiles for qkv
MT = 6  # qkv m-tiles: 2 q, 2 k, 2 v (128 dims each)
NQKV = MT * P  # 768
IT = T // P  # 16 k-token tiles

_CACHE = {}


def _build():
    import concourse.bass as bass  # noqa: F401
    import concourse.mybir as mybir
    import concourse.tile as tile
    from concourse import bacc

    F32 = mybir.dt.float32
    F32R = mybir.dt.float32r
    AF = mybir.ActivationFunctionType

    nc = bacc.Bacc(None, target_bir_lowering=False)
    xT_d = nc.dram_tensor("xT", [D, T], F32R, kind="ExternalInput")
    wqkv_d = nc.dram_tensor("wqkv", [D, NQKV], F32R, kind="ExternalInput")
    bqkv_d = nc.dram_tensor("bqkv2", [P, MT], F32, kind="ExternalInput")
    wproj_d = nc.dram_tensor("wproj", [HD, HPC * D], F32R, kind="ExternalInput")
    masks_d = nc.dram_tensor("masks", [P, 4 * NJ], F32R, kind="ExternalInput")
    ident_d = nc.dram_tensor("ident", [P, P], F32R, kind="ExternalInput")
    out_d = nc.dram_tensor("out", [T, D], F32, kind="ExternalOutput")

    with tile.TileContext(nc) as tc:
        with (
            tc.tile_pool(name="const", bufs=1) as const,
            tc.tile_pool(name="xp", bufs=2) as xp,
            tc.tile_pool(name="qkvps", bufs=2, space="PSUM") as qkvps,
            tc.tile_pool(name="stps", bufs=4, space="PSUM") as stps,
            tc.tile_pool(name="yps", bufs=2, space="PSUM") as yps,
            tc.tile_pool(name="expp", bufs=4) as expp,
            tc.tile_pool(name="recp", bufs=2) as recp,
            tc.tile_pool(name="bcp", bufs=2) as bcp,
            tc.tile_pool(name="outp", bufs=3) as outp,
        ):
            w_sb = const.tile([P, KT, NQKV], F32R)
            bias_sb = const.tile([P, MT], F32)
            wp_sb = const.tile([HD, HPC * D], F32R)
            masks_sb = const.tile([P, 4 * NJ], F32R)
            ident = const.tile([P, P], F32R)
            qkvT_sb = const.tile([P, MT, T], F32R)
            vnat_sb = const.tile([P, 2, IT, 130], F32R)
            yt_sb = const.tile([HD, HPC, T], F32R)

            nc.sync.dma_start(w_sb[:], wqkv_d.rearrange("(kt p) n -> p kt n", p=P))
            nc.sync.dma_start(bias_sb[:], bqkv_d[:])
            nc.sync.dma_start(wp_sb[:], wproj_d[:])
            nc.sync.dma_start(masks_sb[:], masks_d[:])
            nc.sync.dma_start(ident[:], ident_d[:])

            xT_r = xT_d.rearrange("(kt p) t -> p kt t", p=P)

            # ---- Stage 1: qkvT = Wc^T @ xT (+ bias) --------------------
            for j in range(JT):
                xt = xp.tile([P, KT, NJ], F32R)
                nc.sync.dma_start(xt[:], xT_r[:, :, j * NJ : (j + 1) * NJ])
                for m in range(MT):
                    ps = qkvps.tile([P, NJ], F32, tag="mm")
                    for k in range(KT):
                        nc.tensor.matmul(
                            ps[:],
                            w_sb[:, k, m * P : (m + 1) * P],
                            xt[:, k, :],
                            start=(k == 0),
                            stop=(k == KT - 1),
                        )
                    nc.scalar.activation(
                        qkvT_sb[:, m, j * NJ : (j + 1) * NJ],
                        ps[:],
                        AF.Identity,
                        bias=bias_sb[:, m : m + 1],
                    )

            # ---- Stage 2: v -> natural layout (+ ones cols) ------------
            for h2 in range(2):
                nc.gpsimd.memset(vnat_sb[:, h2, :, :].bitcast(F32), 1.0)
                for i in range(IT):
                    pt = qkvps.tile([P, NJ], F32R, tag="mm")
                    nc.tensor.transpose(
                        pt[:, 0:P], qkvT_sb[:, 4 + h2, i * P : (i + 1) * P], ident[:]
                    )
                    nc.vector.tensor_copy(vnat_sb[:, h2, i, 0:HD], pt[:, 0:HD])
                    nc.vector.tensor_copy(vnat_sb[:, h2, i, 65 : 65 + HD], pt[:, HD:P])

            # ---- Stage 3: attention per head-pair ----------------------
            for hp in range(2):
                qm, km = hp, 2 + hp
                for j in range(JT):
                    y_e = yps.tile([P, NJ], F32, tag="y")
                    y_o = yps.tile([P, NJ], F32, tag="y")
                    n_i = 4 * j + 4
                    for i in range(n_i):
                        st_e = stps.tile([P, NJ], F32, tag="st")
                        st_o = stps.tile([P, NJ], F32, tag="st")
                        nc.tensor.matmul(
                            st_e[:],
                            qkvT_sb[0:HD, km, i * P : (i + 1) * P],
                            qkvT_sb[0:HD, qm, j * NJ : (j + 1) * NJ],
                            start=True,
                            stop=True,
                        )
                        nc.tensor.matmul(
                            st_o[:],
                            qkvT_sb[HD:P, km, i * P : (i + 1) * P],
                            qkvT_sb[HD:P, qm, j * NJ : (j + 1) * NJ],
                            start=True,
                            stop=True,
                        )
                        exp_e = expp.tile([P, NJ], F32R, tag="exp")
                        exp_o = expp.tile([P, NJ], F32R, tag="exp")
                        nc.scalar.activation(exp_e[:], st_e[:], AF.Exp)
                        nc.scalar.activation(exp_o[:], st_o[:], AF.Exp)
                        r = i - 4 * j
                        if r >= 0:
                            msl = masks_sb[:, r * NJ : (r + 1) * NJ]
                            nc.vector.tensor_mul(exp_e[:], exp_e[:], msl)
                            nc.vector.tensor_mul(exp_o[:], exp_o[:], msl)
                        nc.tensor.matmul(
                            y_e[0:65, :],
                            vnat_sb[:, hp, i, 0:65],
                            exp_e[:],
                            start=(i == 0),
                            stop=(i == n_i - 1),
                        )
                        nc.tensor.matmul(
                            y_o[0:65, :],
                            vnat_sb[:, hp, i, 65:130],
                            exp_o[:],
                            start=(i == 0),
                            stop=(i == n_i - 1),
                        )
                    for par, y_ps in ((0, y_e), (1, y_o)):
                        rec = recp.tile([P, NJ], F32R)
                        with nc.allow_low_precision(reason="f32r is 4-byte"):
                            nc.vector.reciprocal(rec[64:65, :], y_ps[64:65, :])
                        # broadcast row 64 across 64 partitions via a K=1
                        # matmul; masks_sb[64, 448:512] is an all-ones run.
                        bc_ps = stps.tile([P, NJ], F32, tag="st")
                        nc.tensor.matmul(
                            bc_ps[0:HD, :],
                            masks_sb[64:65, 448:512],
                            rec[64:65, :],
                            start=True,
                            stop=True,
                        )
                        bc = bcp.tile([HD, NJ], F32)
                        nc.scalar.activation(bc[:, :], bc_ps[0:HD, :], AF.Copy)
                        nc.vector.tensor_mul(
                            yt_sb[:, 2 * hp + par, j * NJ : (j + 1) * NJ],
                            y_ps[0:HD, :],
                            bc[:, :],
                        )

            # ---- Stage 4: partial out projection -----------------------
            for qm_i in range(T // P):
                for n in range(2):
                    po = qkvps.tile([P, NJ], F32, tag="mm")
                    for h in range(HPC):
                        nc.tensor.matmul(
                            po[:],
                            yt_sb[:, h, qm_i * P : (qm_i + 1) * P],
                            wp_sb[:, h * D + n * NJ : h * D + (n + 1) * NJ].bitcast(
                                F32R
                            ),
                            start=(h == 0),
                            stop=(h == HPC - 1),
                        )
                    ot = outp.tile([P, NJ], F32)
                    nc.vector.tensor_copy(ot[:], po[:])
                    nc.sync.dma_start(
                        out_d[qm_i * P : (qm_i + 1) * P, n * NJ : (n + 1) * NJ], ot[:]
                    )

    nc.compile()
    return nc


def _prep_inputs(x, Wqkv, bqkv, Wproj):
    """Per-core input maps. Core c -> batch c//4, heads 4*(c%4) .. +4."""
    scale = np.float32(1.0 / np.sqrt(HD))
    masks = np.zeros((P, 4 * NJ), np.float32)
    pp = np.arange(P)[:, None]
    ff = np.arange(NJ)[None, :]
    for r in range(4):
        masks[:, r * NJ : (r + 1) * NJ] = (ff >= P * r + pp).astype(np.float32)

    in_maps = []
    for c in range(N_CORES):
        b, g = divmod(c, HPC)
        cs = slice(256 * g, 256 * g + 256)
        wq = Wqkv[:, 0 * D :][:, cs] * scale
        wk = Wqkv[:, 1 * D : 2 * D][:, cs]
        wv = Wqkv[:, 2 * D : 3 * D][:, cs]
        wqkv_c = np.ascontiguousarray(np.concatenate([wq, wk, wv], axis=1), np.float32)
        bq = bqkv[0 * D :][cs] * scale
        bk = bqkv[1 * D : 2 * D][cs]
        bv = bqkv[2 * D : 3 * D][cs]
        bqkv_c = np.concatenate([bq, bk, bv]).reshape(MT, P).T
        wproj_c = np.concatenate(
            [Wproj[256 * g + HD * h : 256 * g + HD * (h + 1), :] for h in range(HPC)],
            axis=1,
        )
        in_maps.append(
            {
                "xT": np.ascontiguousarray(x[b].T, np.float32),
                "wqkv": wqkv_c,
                "bqkv2": np.ascontiguousarray(bqkv_c, np.float32),
                "wproj": np.ascontiguousarray(wproj_c, np.float32),
                "masks": masks,
                "ident": np.eye(P, dtype=np.float32),
            }
        )
    return in_maps


def kernel(x, Wqkv, bqkv, Wproj, bproj, _trace=False, _trace_out=None):
    from concourse.bass_utils import run_bass_kernel_spmd

    if "nc" not in _CACHE:
        _CACHE["nc"] = _build()
    nc = _CACHE["nc"]

    x = np.asarray(x, np.float32)
    Wqkv = np.asarray(Wqkv, np.float32)
    bqkv = np.asarray(bqkv, np.float32)
    Wproj = np.asarray(Wproj, np.float32)
    bproj = np.asarray(bproj, np.float32)

    in_maps = _prep_inputs(x, Wqkv, bqkv, Wproj)
    res = run_bass_kernel_spmd(
        nc, in_maps, core_ids=list(range(N_CORES)), trace=_trace
    )
    if _trace_out is not None:
        _trace_out.append(res)

    out = np.empty((B, T, D), np.float32)
    for b in range(B):
        acc = res.results[HPC * b]["out"].astype(np.float32)
        for g in range(1, HPC):
            acc = acc + res.results[HPC * b + g]["out"]
        out[b] = acc + bproj[None, :]
    return out


# revision 15
# speedup vs baseline: 1.0144x; 1.0144x over previous
"""Causal self-attention (B=2, T=2048, D=1024, H=16, hd=64) on 8 TRN2 cores.

Sharding: 2 batches x 4 head-groups (4 heads each). Each core computes the
full pipeline for its (batch, head-group): qkv projection (transposed
layout), causal attention, and its partial output projection. The host sums
the 4 per-batch partials (tensor-parallel reduce) and adds bproj.

Device-side layout notes:
 - x is passed pre-transposed (xT [D, T]) so the qkv projection can contract
   over D on the partition dimension.
 - Scores are computed transposed (St = k @ qT, [k_tok, q_tok]) so softmax's
   exp feeds straight into att@v as the moving operand without transposes.
 - Softmax has no max-subtraction (scores are O(6) here, exp is safe) and the
   denominator is produced by augmenting v with a ones column (M=65 matmul).
 - The 1/sqrt(hd) scale is folded into Wq/bq on the host.
"""

import sys

sys.path.insert(0, "/opt/trn_rl_repo")

import numpy as np

B, T, D = 2, 2048, 1024
N_HEAD = 16
HD = 64  # head dim
HPC = 4  # heads per core
N_CORES = 8

P = 128
NJ = 512  # q-slice width
JT = T // NJ  # 4 q-slices
KT = D // P  # 8 contraction tiles for qkv
MT = 6  # qkv m-tiles: 2 q, 2 k, 2 v (128 dims each)
NQKV = MT * P  # 768
IT = T // P  # 16 k-token tiles

_CACHE = {}


def _build():
    import concourse.bass as bass  # noqa: F401
    import concourse.mybir as mybir
    import concourse.tile as tile
    from concourse import bacc

    F32 = mybir.dt.float32
    F32R = mybir.dt.float32r
    AF = mybir.ActivationFunctionType

    nc = bacc.Bacc(None, target_bir_lowering=False)
    xT_d = nc.dram_tensor("xT", [D, T], F32R, kind="ExternalInput")
    wqkv_d = nc.dram_tensor("wqkv", [D, NQKV], F32R, kind="ExternalInput")
    bqkv_d = nc.dram_tensor("bqkv2", [P, MT], F32, kind="ExternalInput")
    wproj_d = nc.dram_tensor("wproj", [HD, HPC * D], F32R, kind="ExternalInput")
    masks_d = nc.dram_tensor("masks", [P, 4 * NJ], F32R, kind="ExternalInput")
    ident_d = nc.dram_tensor("ident", [P, P], F32R, kind="ExternalInput")
    out_d = nc.dram_tensor("out", [T, D], F32, kind="ExternalOutput")

    with tile.TileContext(nc) as tc:
        with (
            tc.tile_pool(name="const", bufs=1) as const,
            tc.tile_pool(name="xp", bufs=2) as xp,
            tc.tile_pool(name="stps", bufs=4, space="PSUM") as stps,
            tc.tile_pool(name="yps", bufs=4, space="PSUM") as yps,
            tc.tile_pool(name="expp", bufs=4) as expp,
            tc.tile_pool(name="recp", bufs=2) as recp,
            tc.tile_pool(name="bcp", bufs=2) as bcp,
            tc.tile_pool(name="outp", bufs=3) as outp,
        ):
            w_sb = const.tile([P, KT, NQKV], F32R)
            bias_sb = const.tile([P, MT], F32)
            wp_sb = const.tile([HD, HPC * D], F32R)
            masks_sb = const.tile([P, 4 * NJ], F32R)
            ident = const.tile([P, P], F32R)
            qkvT_sb = const.tile([P, MT, T], F32R)
            vnat_sb = const.tile([P, 2, IT, 130], F32R)
            yt_sb = const.tile([HD, HPC, T], F32R)

            nc.sync.dma_start(w_sb[:], wqkv_d.rearrange("(kt p) n -> p kt n", p=P))
            nc.sync.dma_start(bias_sb[:], bqkv_d[:])
            nc.sync.dma_start(wp_sb[:], wproj_d[:])
            nc.sync.dma_start(masks_sb[:], masks_d[:])
            nc.sync.dma_start(ident[:], ident_d[:])

            xT_r = xT_d.rearrange("(kt p) t -> p kt t", p=P)

            # ---- Stage 1: qkvT = Wc^T @ xT (+ bias) --------------------
            for j in range(JT):
                xt = xp.tile([P, KT, NJ], F32R)
                nc.sync.dma_start(xt[:], xT_r[:, :, j * NJ : (j + 1) * NJ])
                for m in range(MT):
                    ps = stps.tile([P, NJ], F32, tag="st")
                    for k in range(KT):
                        nc.tensor.matmul(
                            ps[:],
                            w_sb[:, k, m * P : (m + 1) * P],
                            xt[:, k, :],
                            start=(k == 0),
                            stop=(k == KT - 1),
                        )
                    nc.scalar.activation(
                        qkvT_sb[:, m, j * NJ : (j + 1) * NJ],
                        ps[:],
                        AF.Identity,
                        bias=bias_sb[:, m : m + 1],
                    )

            # ---- Stage 2: v -> natural layout (+ ones cols) ------------
            for h2 in range(2):
                nc.gpsimd.memset(vnat_sb[:, h2, :, :].bitcast(F32), 1.0)
                for i in range(IT):
                    pt = stps.tile([P, NJ], F32R, tag="st")
                    nc.tensor.transpose(
                        pt[:, 0:P], qkvT_sb[:, 4 + h2, i * P : (i + 1) * P], ident[:]
                    )
                    nc.vector.tensor_copy(vnat_sb[:, h2, i, 0:HD], pt[:, 0:HD])
                    nc.vector.tensor_copy(vnat_sb[:, h2, i, 65 : 65 + HD], pt[:, HD:P])

            # ---- Stage 3: attention per head-pair ----------------------
            # Software-pipelined: St(i) is issued before Y(i-1) so ACT's
            # exp(i-1) overlaps the PE's St(i); normalization of slice (hp,j)
            # is deferred into slice (hp,j)+1's loop so the reciprocal's
            # latency hides behind matmul work.

            def emit_norm(pend):
                hp_, j_, ye_, yo_ = pend
                for par, y_ps in ((0, ye_), (1, yo_)):
                    rec = recp.tile([P, NJ], F32R)
                    with nc.allow_low_precision(reason="f32r is 4-byte"):
                        nc.vector.reciprocal(rec[64:65, :], y_ps[64:65, :])
                    # broadcast row 64 across 64 partitions via a K=1
                    # matmul; masks_sb[64, 448:512] is an all-ones run.
                    bc_ps = stps.tile([P, NJ], F32, tag="st")
                    nc.tensor.matmul(
                        bc_ps[0:HD, :],
                        masks_sb[64:65, 448:512],
                        rec[64:65, :],
                        start=True,
                        stop=True,
                    )
                    bc = bcp.tile([HD, NJ], F32)
                    nc.scalar.activation(bc[:, :], bc_ps[0:HD, :], AF.Copy)
                    nc.vector.tensor_mul(
                        yt_sb[:, 2 * hp_ + par, j_ * NJ : (j_ + 1) * NJ],
                        y_ps[0:HD, :],
                        bc[:, :],
                    )

            pend_norm = None
            for hp in range(2):
                qm, km = hp, 2 + hp
                for j in range(JT):
                    y_e = yps.tile([P, NJ], F32, tag="y")
                    y_o = yps.tile([P, NJ], F32, tag="y")
                    n_i = 4 * j + 4
                    prev = None
                    for i in range(n_i):
                        st_e = stps.tile([P, NJ], F32, tag="st")
                        st_o = stps.tile([P, NJ], F32, tag="st")
                        nc.tensor.matmul(
                            st_e[:],
                            qkvT_sb[0:HD, km, i * P : (i + 1) * P],
                            qkvT_sb[0:HD, qm, j * NJ : (j + 1) * NJ],
                            start=True,
                            stop=True,
                        )
                        nc.tensor.matmul(
                            st_o[:],
                            qkvT_sb[HD:P, km, i * P : (i + 1) * P],
                            qkvT_sb[HD:P, qm, j * NJ : (j + 1) * NJ],
                            start=True,
                            stop=True,
                        )
                        exp_e = expp.tile([P, NJ], F32R, tag="exp")
                        exp_o = expp.tile([P, NJ], F32R, tag="exp")
                        r = i - 4 * j
                        if r < 0:
                            nc.scalar.activation(exp_e[:], st_e[:], AF.Exp)
                            nc.scalar.activation(exp_o[:], st_o[:], AF.Exp)
                        else:
                            # diag block: cols [0, 128r) are fully above the
                            # causal line -> zero; cols [128r, 128r+128) are
                            # triangular; the rest is fully kept.
                            c0 = P * r
                            for ex, st in ((exp_e, st_e), (exp_o, st_o)):
                                if c0 > 0:
                                    nc.gpsimd.memset(
                                        ex[:, 0:c0].bitcast(F32), 0.0
                                    )
                                nc.scalar.activation(
                                    ex[:, c0:NJ], st[:, c0:NJ], AF.Exp
                                )
                                nc.vector.tensor_mul(
                                    ex[:, c0 : c0 + P],
                                    ex[:, c0 : c0 + P],
                                    masks_sb[:, 0:P],
                                )
                        if i == 1 and pend_norm is not None:
                            emit_norm(pend_norm)
                            pend_norm = None
                        if prev is not None:
                            pi, pe, po_ = prev
                            nc.tensor.matmul(
                                y_e[0:65, :],
                                vnat_sb[:, hp, pi, 0:65],
                                pe[:],
                                start=(pi == 0),
                                stop=False,
                            )
                            nc.tensor.matmul(
                                y_o[0:65, :],
                                vnat_sb[:, hp, pi, 65:130],
                                po_[:],
                                start=(pi == 0),
                                stop=False,
                            )
                        prev = (i, exp_e, exp_o)
                    pi, pe, po_ = prev
                    nc.tensor.matmul(
                        y_e[0:65, :],
                        vnat_sb[:, hp, pi, 0:65],
                        pe[:],
                        start=(pi == 0),
                        stop=True,
                    )
                    nc.tensor.matmul(
                        y_o[0:65, :],
                        vnat_sb[:, hp, pi, 65:130],
                        po_[:],
                        start=(pi == 0),
                        stop=True,
                    )
                    pend_norm = (hp, j, y_e, y_o)
            emit_norm(pend_norm)

            # ---- Stage 4: partial out projection -----------------------
            for qm_i in range(T // P):
                for n in range(2):
                    po = stps.tile([P, NJ], F32, tag="st")
                    for h in range(HPC):
                        nc.tensor.matmul(
                            po[:],
                            yt_sb[:, h, qm_i * P : (qm_i + 1) * P],
                            wp_sb[:, h * D + n * NJ : h * D + (n + 1) * NJ].bitcast(
                                F32R
                            ),
                            start=(h == 0),
                            stop=(h == HPC - 1),
                        )
                    ot = outp.tile([P, NJ], F32)
                    nc.vector.tensor_copy(ot[:], po[:])
                    nc.sync.dma_start(
                        out_d[qm_i * P : (qm_i + 1) * P, n * NJ : (n + 1) * NJ], ot[:]
                    )

    nc.compile()
    return nc


def _prep_inputs(x, Wqkv, bqkv, Wproj):
    """Per-core input maps. Core c -> batch c//4, heads 4*(c%4) .. +4."""
    scale = np.float32(1.0 / np.sqrt(HD))
    masks = np.zeros((P, 4 * NJ), np.float32)
    pp = np.arange(P)[:, None]
    ff = np.arange(NJ)[None, :]
    for r in range(4):
        masks[:, r * NJ : (r + 1) * NJ] = (ff >= P * r + pp).astype(np.float32)

    in_maps = []
    for c in range(N_CORES):
        b, g = divmod(c, HPC)
        cs = slice(256 * g, 256 * g + 256)
        wq = Wqkv[:, 0 * D :][:, cs] * scale
        wk = Wqkv[:, 1 * D : 2 * D][:, cs]
        wv = Wqkv[:, 2 * D : 3 * D][:, cs]
        wqkv_c = np.ascontiguousarray(np.concatenate([wq, wk, wv], axis=1), np.float32)
        bq = bqkv[0 * D :][cs] * scale
        bk = bqkv[1 * D : 2 * D][cs]
        bv = bqkv[2 * D : 3 * D][cs]
        bqkv_c = np.concatenate([bq, bk, bv]).reshape(MT, P).T
        wproj_c = np.concatenate(
            [Wproj[256 * g + HD * h : 256 * g + HD * (h + 1), :] for h in range(HPC)],
            axis=1,
        )
        in_maps.append(
            {
                "xT": np.ascontiguousarray(x[b].T, np.float32),
                "wqkv": wqkv_c,
                "bqkv2": np.ascontiguousarray(bqkv_c, np.float32),
                "wproj": np.ascontiguousarray(wproj_c, np.float32),
                "masks": masks,
                "ident": np.eye(P, dtype=np.float32),
            }
        )
    return in_maps


def kernel(x, Wqkv, bqkv, Wproj, bproj, _trace=False, _trace_out=None):
    from concourse.bass_utils import run_bass_kernel_spmd

    if "nc" not in _CACHE:
        _CACHE["nc"] = _build()
    nc = _CACHE["nc"]

    x = np.asarray(x, np.float32)
    Wqkv = np.asarray(Wqkv, np.float32)
    bqkv = np.asarray(bqkv, np.float32)
    Wproj = np.asarray(Wproj, np.float32)
    bproj = np.asarray(bproj, np.float32)

    in_maps = _prep_inputs(x, Wqkv, bqkv, Wproj)
    res = run_bass_kernel_spmd(
        nc, in_maps, core_ids=list(range(N_CORES)), trace=_trace
    )
    if _trace_out is not None:
        _trace_out.append(res)

    out = np.empty((B, T, D), np.float32)
    for b in range(B):
        acc = res.results[HPC * b]["out"].astype(np.float32)
        for g in range(1, HPC):
            acc = acc + res.results[HPC * b + g]["out"]
        out[b] = acc + bproj[None, :]
    return out
